# revision 1
# baseline (speedup 1.0000x reference)
"""GIN-style GNN message passing kernel for Trainium2 (8 NeuronCores).

Strategy:
  - Host: sort edges by dst, shard edges across cores at node-range
    boundaries (each core owns N/C destination nodes -> no collectives).
    Edges are further split into 4 streams by (src0 < H, src1 < H) with
    H = 32768 so every dma_gather row index fits in signed int16.
  - Device (per core, SPMD):
      phase 1: tables h0 = x@W0, h1 = x@W1  (bf16, PE) -> DRAM scratch
      phase 2: per stream region: big dma_gather calls for h0[src0], h1[src1]
      phase 3: edge embedding a@Wa + (b0+b1+ba) via block-diag matmuls
      phase 4: msg = relu(h0g + h1g + aemb)
      phase 5: segment-sum as one-hot matmuls -> per-(stream, window) PSUM,
               accumulated into an SBUF agg [128, NW*128] fp32
      phase 6: h = (1+eps)*x + agg ; relu(h@W_in + b_in) @ W_out + b_out
               in transposed layout (biases land on partitions)
  - Host: transpose + concat per-core outputs.
"""

import math
from dataclasses import dataclass, field

import numpy as np
import ml_dtypes

import concourse.bass as bass
import concourse.mybir as mybir
import concourse.tile as tile
from concourse import bacc
from concourse import bass_utils

BF16 = mybir.dt.bfloat16
F32 = mybir.dt.float32
I16 = mybir.dt.int16
NBF = ml_dtypes.bfloat16

P = 128


@dataclass
class Meta:
    C: int
    N: int
    D: int
    DA: int
    NPC: int
    NW: int
    HALF: int                  # stream split threshold (int16-safe)
    TPW: list = field(default_factory=list)    # tiles per window, per stream
    R_pad: list = field(default_factory=list)  # padded region tiles, per stream
    T_alloc: int = 0           # sum of R_pad
    GK: int = 16               # tiles per gather call
    NP: int = 0
    NT: int = 0


def _host_prep(x, index, a, W0, b0, W1, b1, Wa, ba, eps, W_in, b_in, W_out,
               b_out, C=8, gk=8, half=32768):
    x = np.asarray(x, np.float32)
    a = np.asarray(a, np.float32)
    N, D = x.shape
    E = index.shape[1]
    DA = a.shape[1]
    assert D == P
    NPC = math.ceil(N / C)
    NW = math.ceil(NPC / P)

    dst = np.asarray(index[0], np.int64)
    s0 = np.asarray(index[1], np.int64)
    s1 = np.asarray(index[2], np.int64)

    st = 2 * (s0 >= half) + (s1 >= half)
    c_of = dst // NPC
    rel = dst - c_of * NPC
    w_of = rel // P
    off = (rel - w_of * P).astype(np.float32)

    # order edges by (core, stream, window)
    order = np.lexsort((w_of, st, c_of))
    dsts, s0s, s1s, a_s = dst[order], s0[order], s1[order], a[order]
    sts, cs, ws, offs = st[order], c_of[order], w_of[order], off[order]

    # counts per (c, st, w)
    key = (cs * 4 + sts) * NW + ws
    counts = np.bincount(key, minlength=C * 4 * NW).reshape(C, 4, NW)
    TPW = [max(0, int(math.ceil(counts[:, s, :].max() / P))) for s in range(4)]
    R = [NW * t for t in TPW]
    R_pad = [math.ceil(r / 4) * 4 for r in R]
    T_alloc = sum(R_pad)
    base = np.cumsum([0] + R_pad[:-1])
    NP = math.ceil(N / P) * P

    meta = Meta(C=C, N=N, D=D, DA=DA, NPC=NPC, NW=NW, HALF=half, TPW=TPW,
                R_pad=R_pad, T_alloc=T_alloc, GK=gk, NP=NP, NT=NP // P)

    excl = np.concatenate(([0], np.cumsum(counts.ravel())))[:-1]
    rank = np.arange(E) - excl[key]
    slot = (base[sts] + ws * np.array(TPW)[sts]) * P + sts * 0 + rank \
        + (np.zeros_like(rank))
    slot = (base[sts] + ws * np.array(TPW)[sts]) * P + rank

    eps_f = float(np.asarray(eps).reshape(-1)[0])
    KA = DA + 1

    xT_all = np.zeros((P, NP), NBF)
    xT_all[:, :N] = x.T.astype(NBF)
    w01 = np.concatenate([W0, W1], axis=1).astype(NBF)
    bsum = (np.asarray(b0) + np.asarray(b1) + np.asarray(ba)).astype(np.float32)
    wa_aug = np.concatenate([np.asarray(Wa, np.float32), bsum[None, :]], axis=0)
    wabd = np.zeros((4 * KA, 4 * P), np.float32)
    for u in range(4):
        wabd[u * KA:(u + 1) * KA, u * P:(u + 1) * P] = wa_aug
    wabd = wabd.astype(NBF)
    iota = np.broadcast_to(np.arange(P, dtype=np.float32), (P, P)).astype(NBF)
    w_in_b = np.asarray(W_in, np.float32).astype(NBF)
    w_out_b = np.asarray(W_out, np.float32).astype(NBF)
    b_in_c = np.asarray(b_in, np.float32).reshape(P, 1)
    b_out_c = np.asarray(b_out, np.float32).reshape(P, 1)

    def pack16(vals):
        # flat position j -> [j % 16, j // 16], replicated to 128 partitions
        arr = np.ascontiguousarray(vals.reshape(-1, 16).T)
        return np.tile(arr, (8, 1))

    in_maps = []
    for c in range(C):
        m = cs == c
        s0_pad = np.zeros(T_alloc * P, np.int64)
        s1_pad = np.zeros(T_alloc * P, np.int64)
        dof_pad = np.full(T_alloc * P, -1.0, np.float32)
        a_pad = np.zeros((T_alloc * P, KA), np.float32)
        a_pad[:, DA] = 1.0
        sl = slot[m]
        s0_pad[sl] = s0s[m]
        s1_pad[sl] = s1s[m]
        dof_pad[sl] = offs[m]
        a_pad[sl, :DA] = a_s[m]

        # per-region index bias: stream bit 2 -> s0 in hi half; bit 1 -> s1 hi
        for s in range(4):
            lo = base[s] * P
            hi = lo + R_pad[s] * P
            if s >= 2:
                s0_pad[lo:hi] = np.maximum(s0_pad[lo:hi] - half, 0)
            if s % 2 == 1:
                s1_pad[lo:hi] = np.maximum(s1_pad[lo:hi] - half, 0)
        idx0 = pack16(s0_pad.astype(np.int16))
        idx1 = pack16(s1_pad.astype(np.int16))
        dofT = np.ascontiguousarray(dof_pad.reshape(T_alloc, P).T)

        NG = T_alloc // 4
        a3 = a_pad.reshape(NG, 4, P, KA)
        slabs = np.ascontiguousarray(
            a3.transpose(0, 1, 3, 2).reshape(NG, 4 * KA, P)).astype(NBF)

        lo_n = c * NPC
        hi_n = min((c + 1) * NPC, N)
        xtn = np.zeros((P, NW * P), np.float32)
        xtn[:, :hi_n - lo_n] = (1.0 + eps_f) * x[lo_n:hi_n].T

        in_maps.append({
            "xT_all": xT_all, "w01": w01, "wabd": wabd, "iota": iota,
            "slabs": slabs, "idx0": idx0, "idx1": idx1, "dofT": dofT,
            "xtn": xtn, "w_in": w_in_b, "w_out": w_out_b,
            "b_in": b_in_c, "b_out": b_out_c,
        })
    return meta, in_maps


def _build(meta: Meta):
    nc = bacc.Bacc("TRN2", target_bir_lowering=False, debug=False,
                   enable_asserts=False, num_devices=meta.C)
    KA = meta.DA + 1
    T_alloc = meta.T_alloc
    NG = T_alloc // 4

    xT_all = nc.dram_tensor("xT_all", [P, meta.NP], BF16, kind="ExternalInput")
    w01_d = nc.dram_tensor("w01", [P, 2 * P], BF16, kind="ExternalInput")
    wabd_d = nc.dram_tensor("wabd", [4 * KA, 4 * P], BF16, kind="ExternalInput")
    iota_d = nc.dram_tensor("iota", [P, P], BF16, kind="ExternalInput")
    slabs_d = nc.dram_tensor("slabs", [NG, 4 * KA, P], BF16,
                             kind="ExternalInput")
    idx0_d = nc.dram_tensor("idx0", [P, T_alloc * 8], I16, kind="ExternalInput")
    idx1_d = nc.dram_tensor("idx1", [P, T_alloc * 8], I16, kind="ExternalInput")
    dofT_d = nc.dram_tensor("dofT", [P, T_alloc], F32, kind="ExternalInput")
    xtn_d = nc.dram_tensor("xtn", [P, meta.NW * P], F32, kind="ExternalInput")
    w_in_d = nc.dram_tensor("w_in", [P, P], BF16, kind="ExternalInput")
    w_out_d = nc.dram_tensor("w_out", [P, P], BF16, kind="ExternalInput")
    b_in_d = nc.dram_tensor("b_in", [P, 1], F32, kind="ExternalInput")
    b_out_d = nc.dram_tensor("b_out", [P, 1], F32, kind="ExternalInput")

    h0_tab = nc.dram_tensor("h0_tab", [meta.NP, P], BF16, kind="Internal")
    h1_tab = nc.dram_tensor("h1_tab", [meta.NP, P], BF16, kind="Internal")
    yT_d = nc.dram_tensor("yT", [P, meta.NW * P], F32, kind="ExternalOutput")

    GK = meta.GK
    H = meta.HALF

    with tile.TileContext(nc) as tc:
        with (
            tc.tile_pool(name="const", bufs=1) as cpool,
            tc.tile_pool(name="xt", bufs=4) as xtp,
            tc.tile_pool(name="tabsb", bufs=4) as tabsb,
            tc.tile_pool(name="slab", bufs=4) as slabp,
            tc.tile_pool(name="hg", bufs=3) as hgp,
            tc.tile_pool(name="ab", bufs=2) as abp,
            tc.tile_pool(name="msg", bufs=8) as msgp,
            tc.tile_pool(name="oh", bufs=8) as ohp,
            tc.tile_pool(name="mlp", bufs=4) as mlpp,
            tc.tile_pool(name="ps_misc", bufs=3, space="PSUM") as psm,
            tc.tile_pool(name="ps_aemb", bufs=2, space="PSUM") as psa,
            tc.tile_pool(name="ps_agg", bufs=2, space="PSUM") as psg,
        ):
            idx0 = cpool.tile([P, T_alloc * 8], I16, tag="idx0")
            idx1 = cpool.tile([P, T_alloc * 8], I16, tag="idx1")
            dofT = cpool.tile([P, T_alloc], F32, tag="dofT")
            iota = cpool.tile([P, P], BF16, tag="iota")
            w01 = cpool.tile([P, 2 * P], BF16, tag="w01")
            wabd = cpool.tile([4 * KA, 4 * P], BF16, tag="wabd")
            xtn = cpool.tile([P, meta.NW * P], F32, tag="xtn")
            w_in = cpool.tile([P, P], BF16, tag="w_in")
            w_out = cpool.tile([P, P], BF16, tag="w_out")
            b_in = cpool.tile([P, 1], F32, tag="b_in")
            b_out = cpool.tile([P, 1], F32, tag="b_out")
            agg_sb = cpool.tile([P, meta.NW * P], F32, tag="agg_sb")
            for t, d in [(idx0, idx0_d), (idx1, idx1_d), (dofT, dofT_d),
                         (iota, iota_d), (w01, w01_d), (wabd, wabd_d),
                         (xtn, xtn_d), (w_in, w_in_d), (w_out, w_out_d),
                         (b_in, b_in_d), (b_out, b_out_d)]:
                nc.sync.dma_start(t[:], d[:])

            # ---- phase 1: tables ----
            for i in range(meta.NT):
                xt = xtp.tile([P, P], BF16, tag="xt")
                nc.sync.dma_start(xt[:], xT_all[:, i * P:(i + 1) * P])
                ps = psm.tile([P, 2 * P], F32, tag="pm")
                nc.tensor.matmul(ps[:], xt[:], w01[:], start=True, stop=True)
                hsb = tabsb.tile([P, 2 * P], BF16, tag="hsb")
                nc.any.tensor_copy(hsb[:], ps[:])
                nc.sync.dma_start(h0_tab[i * P:(i + 1) * P, :], hsb[:, 0:P])
                nc.sync.dma_start(h1_tab[i * P:(i + 1) * P, :], hsb[:, P:2 * P])

            # ---- phases 2-6, stream-major ----
            def finalize(w):
                hbf = mlpp.tile([P, P], BF16, tag="hbf")
                nc.vector.tensor_add(hbf[:], agg_sb[:, w * P:(w + 1) * P],
                                     xtn[:, w * P:(w + 1) * P])
                z1 = psm.tile([P, P], F32, tag="pm")
                nc.tensor.matmul(z1[:], w_in[:], hbf[:], start=True, stop=True)
                z1b = mlpp.tile([P, P], BF16, tag="z1b")
                nc.scalar.activation(z1b[:], z1[:],
                                     mybir.ActivationFunctionType.Relu,
                                     bias=b_in[:, 0:1])
                z2 = psm.tile([P, P], F32, tag="pm")
                nc.tensor.matmul(z2[:], w_out[:], z1b[:], start=True, stop=True)
                ysb = mlpp.tile([P, P], F32, tag="ysb")
                nc.vector.tensor_scalar(ysb[:], z2[:], b_out[:, 0:1], None,
                                        op0=mybir.AluOpType.add)
                nc.sync.dma_start(yT_d[:, w * P:(w + 1) * P], ysb[:])

            flat_base = 0
            n_live = 0  # streams with edges
            live = [s for s in range(4) if meta.TPW[s] > 0]
            for s in range(4):
                Rp = meta.R_pad[s]
                if Rp == 0:
                    continue
                Rr = meta.NW * meta.TPW[s]
                h0v = h0_tab[H:, :] if s >= 2 else h0_tab[:, :]
                h1v = h1_tab[H:, :] if s % 2 == 1 else h1_tab[:, :]
                first = (s == live[0])
                last = (s == live[-1])
                agg = None
                for c0 in range(0, Rp, GK):
                    k = min(GK, Rp - c0)
                    q0 = (flat_base + c0) * 8  # idx col offset (pos/16)
                    hg0 = hgp.tile([P, GK, P], BF16, tag="hg0")
                    nc.gpsimd.dma_gather(
                        out_ap=hg0[:, :k, :], in_ap=h0v,
                        idxs_ap=idx0[:, q0:q0 + k * 8],
                        num_idxs=k * P, num_idxs_reg=k * P, elem_size=P)
                    hg1 = hgp.tile([P, GK, P], BF16, tag="hg1")
                    nc.gpsimd.dma_gather(
                        out_ap=hg1[:, :k, :], in_ap=h1v,
                        idxs_ap=idx1[:, q0:q0 + k * 8],
                        num_idxs=k * P, num_idxs_reg=k * P, elem_size=P)
                    ab = abp.tile([P, GK * P], BF16, tag="ab")
                    for g in range(k // 4):
                        gf = (flat_base + c0) // 4 + g
                        slab = slabp.tile([4 * KA, P], BF16, tag="slab")
                        nc.sync.dma_start(slab[:], slabs_d[gf, :, :])
                        aps = psa.tile([P, 4 * P], F32, tag="aemb")
                        nc.tensor.matmul(aps[:], slab[:], wabd[:],
                                         start=True, stop=True)
                        nc.any.tensor_copy(ab[:, g * 4 * P:(g + 1) * 4 * P],
                                           aps[:])
                    for t in range(k):
                        pos = c0 + t
                        if pos >= Rr:
                            break
                        tau = flat_base + pos
                        w, t_in_w = divmod(pos, meta.TPW[s])
                        pre = msgp.tile([P, P], BF16, tag="pre")
                        nc.vector.tensor_add(pre[:], hg0[:, t, :], hg1[:, t, :])
                        pre2 = msgp.tile([P, P], BF16, tag="pre2")
                        nc.vector.tensor_add(pre2[:], pre[:],
                                             ab[:, t * P:(t + 1) * P])
                        msg = msgp.tile([P, P], BF16, tag="msg")
                        nc.scalar.activation(msg[:], pre2[:],
                                             mybir.ActivationFunctionType.Relu)
                        oh = ohp.tile([P, P], BF16, tag="oh")
                        nc.vector.tensor_scalar(oh[:], iota[:],
                                                dofT[:, tau:tau + 1], None,
                                                op0=mybir.AluOpType.is_equal)
                        if t_in_w == 0:
                            agg = psg.tile([P, P], F32, tag="agg")
                        nc.tensor.matmul(agg[:], msg[:], oh[:],
                                         start=(t_in_w == 0),
                                         stop=(t_in_w == meta.TPW[s] - 1),
                                         skip_group_check=True)
                        if t_in_w == meta.TPW[s] - 1:
                            sl = slice(w * P, (w + 1) * P)
                            if first:
                                nc.any.tensor_copy(agg_sb[:, sl], agg[:])
                            else:
                                nc.vector.tensor_add(agg_sb[:, sl],
                                                     agg_sb[:, sl], agg[:])
                            if last:
                                finalize(w)
                flat_base += Rp

    nc.compile()
    return nc


def run(inputs: dict, C=8, gk=8, half=32768, trace=False):
    meta, in_maps = _host_prep(
        inputs["x"], inputs["index"], inputs["a"], inputs["W0"], inputs["b0"],
        inputs["W1"], inputs["b1"], inputs["Wa"], inputs["ba"], inputs["eps"],
        inputs["W_in"], inputs["b_in"], inputs["W_out"], inputs["b_out"],
        C=C, gk=gk, half=half)
    nc = _build(meta)
    res = bass_utils.run_bass_kernel_spmd(nc, in_maps, core_ids=list(range(C)),
                                          trace=trace)
    N = meta.N
    out = np.empty((N, P), np.float32)
    for c in range(C):
        lo = c * meta.NPC
        hi = min((c + 1) * meta.NPC, N)
        out[lo:hi] = res.results[c]["yT"].T[:hi - lo]
    return out, res, meta, in_maps, nc


def kernel(**inputs) -> np.ndarray:
    out, _, _, _, _ = run(inputs)
    return out



# revision 7
# speedup vs baseline: 3.3358x; 3.3358x over previous
"""GIN-style GNN message passing kernel for Trainium2 (8 NeuronCores).

Strategy (v2):
  - Host: sort edges by (core, stream, window) with windows of WIN=256 dst
    nodes; streams split (src0>=H, src1>=H) so gather indices fit int16.
  - Device (per core, SPMD):
      phase 1: fused table tab[n] = [x@W0 | x@W1][n] (bf16, 256B x 2 halves)
               built lo-half first so stream 0 gathers start early
      phase 2 per stream region, GK=32-tile gather groups:
        - dma_gather h0[src0], h1[src1] from the fused table (elem_step=256),
          cycling SWDGE queues 0-3 (4 Q7 cpu pairs generate descriptors in
          parallel -> ~4x gather throughput)
        - add1 = hg0 + hg1 in ONE vector op per group
        - per 4-tile slab: aemb = slab @ wabd (block-diag, +b0+b1+ba);
          pre = add1 + aemb (vector, reads PSUM); msg = relu(pre) (scalar)
        - one-hot oh[e, n] = (iota[n] == dof[e]) built 4 tiles/op via
          stride-0 broadcast tensor_tensor is_equal
        - scatter: agg[f, n] += msg.T @ oh accumulated in PSUM per window
        - window done: fold into agg_sb (SBUF f32); after last stream:
          finalize = GIN MLP in transposed layout, biases via scalar.activation
  - Host: transpose + concat per-core outputs.
"""

import math
from dataclasses import dataclass, field

import numpy as np
import ml_dtypes

import concourse.bass as bass
import concourse.mybir as mybir
import concourse.tile as tile
from concourse import bacc
from concourse import bass_utils

BF16 = mybir.dt.bfloat16
F32 = mybir.dt.float32
I16 = mybir.dt.int16
NBF = ml_dtypes.bfloat16

P = 128


@dataclass
class Meta:
    C: int
    N: int
    D: int
    DA: int
    NPC: int
    NW: int
    WIN: int
    HALF: int
    TPW: list = field(default_factory=list)    # tiles per window, per stream
    R: list = field(default_factory=list)      # region tiles, per stream
    R_pad: list = field(default_factory=list)  # padded (mult of 4)
    T_alloc: int = 0
    GK: int = 32
    NP: int = 0
    NT: int = 0
    NLO: int = 0   # lo-half table rows


def _host_prep(x, index, a, W0, b0, W1, b1, Wa, ba, eps, W_in, b_in, W_out,
               b_out, C=8, gk=32, half=32768, win=256):
    x = np.asarray(x, np.float32)
    a = np.asarray(a, np.float32)
    N, D = x.shape
    E = index.shape[1]
    DA = a.shape[1]
    assert D == P
    NPC = math.ceil(N / C)
    NW = math.ceil(NPC / win)

    dst = np.asarray(index[0], np.int64)
    s0 = np.asarray(index[1], np.int64)
    s1 = np.asarray(index[2], np.int64)

    st = 2 * (s0 >= half) + (s1 >= half)
    c_of = dst // NPC
    rel = dst - c_of * NPC
    w_of = rel // win
    off = (rel - w_of * win).astype(np.float32)

    order = np.lexsort((w_of, st, c_of))
    dsts, s0s, s1s, a_s = dst[order], s0[order], s1[order], a[order]
    sts, cs, ws, offs = st[order], c_of[order], w_of[order], off[order]

    key = (cs * 4 + sts) * NW + ws
    counts = np.bincount(key, minlength=C * 4 * NW).reshape(C, 4, NW)
    TPW = [max(0, int(math.ceil(counts[:, s, :].max() / P))) for s in range(4)]
    R = [NW * t for t in TPW]
    R_pad = [math.ceil(r / 4) * 4 for r in R]
    T_alloc = sum(R_pad)
    base = np.cumsum([0] + R_pad[:-1])
    NP = math.ceil(N / P) * P
    NLO = min(half, NP)

    meta = Meta(C=C, N=N, D=D, DA=DA, NPC=NPC, NW=NW, WIN=win, HALF=half,
                TPW=TPW, R=R, R_pad=R_pad, T_alloc=T_alloc, GK=gk, NP=NP,
                NT=NP // P, NLO=NLO)

    excl = np.concatenate(([0], np.cumsum(counts.ravel())))[:-1]
    rank = np.arange(E) - excl[key]
    slot = (base[sts] + ws * np.array(TPW)[sts]) * P + rank

    eps_f = float(np.asarray(eps).reshape(-1)[0])
    KA = DA + 1

    xT_all = np.zeros((P, NP), NBF)
    xT_all[:, :N] = x.T.astype(NBF)
    w01 = np.concatenate([W0, W1], axis=1).astype(NBF)
    ident = np.eye(P, dtype=np.float32).astype(NBF)
    bsum = (np.asarray(b0) + np.asarray(b1) + np.asarray(ba)).astype(np.float32)
    wa_aug = np.concatenate([np.asarray(Wa, np.float32), bsum[None, :]], axis=0)
    wabd = np.zeros((4 * KA, 4 * P), np.float32)
    for u in range(4):
        wabd[u * KA:(u + 1) * KA, u * P:(u + 1) * P] = wa_aug
    wabd = wabd.astype(NBF)
    iota = np.broadcast_to(np.arange(win, dtype=np.float32), (P, win)
                           ).astype(NBF)
    w_in_b = np.asarray(W_in, np.float32).astype(NBF)
    w_out_b = np.asarray(W_out, np.float32).astype(NBF)
    b_in_c = np.asarray(b_in, np.float32).reshape(P, 1)
    b_out_c = np.asarray(b_out, np.float32).reshape(P, 1)

    def pack16(vals):
        arr = np.ascontiguousarray(vals.reshape(-1, 16).T)
        return np.tile(arr, (8, 1))

    in_maps = []
    for c in range(C):
        m = cs == c
        s0_pad = np.zeros(T_alloc * P, np.int64)
        s1_pad = np.zeros(T_alloc * P, np.int64)
        dof_pad = np.full(T_alloc * P, -1.0, np.float32)
        a_pad = np.zeros((T_alloc * P, KA), np.float32)
        a_pad[:, DA] = 1.0
        sl = slot[m]
        s0_pad[sl] = s0s[m]
        s1_pad[sl] = s1s[m]
        dof_pad[sl] = offs[m]
        a_pad[sl, :DA] = a_s[m]

        for s in range(4):
            lo = base[s] * P
            hi = lo + R_pad[s] * P
            if s >= 2:
                s0_pad[lo:hi] = np.maximum(s0_pad[lo:hi] - half, 0)
            if s % 2 == 1:
                s1_pad[lo:hi] = np.maximum(s1_pad[lo:hi] - half, 0)
        idx0 = pack16(s0_pad.astype(np.int16))
        idx1 = pack16(s1_pad.astype(np.int16))
        dofT = np.ascontiguousarray(
            dof_pad.reshape(T_alloc, P).T).astype(NBF)

        # slabs: [4*KA, (T_alloc//4) * P] bf16, group gi at cols gi*P
        NG = T_alloc // 4
        a3 = a_pad.reshape(NG, 4, P, KA)
        slabs = np.ascontiguousarray(
            a3.transpose(1, 3, 0, 2).reshape(4 * KA, NG * P)).astype(NBF)

        lo_n = c * NPC
        hi_n = min((c + 1) * NPC, N)
        xtn = np.zeros((P, NW * win), np.float32)
        xtn[:, :hi_n - lo_n] = (1.0 + eps_f) * x[lo_n:hi_n].T

        in_maps.append({
            "xT_all": xT_all, "w01": w01, "wabd": wabd, "iota": iota,
            "ident": ident,
            "slabs": slabs, "idx0": idx0, "idx1": idx1, "dofT": dofT,
            "xtn": xtn, "w_in": w_in_b, "w_out": w_out_b,
            "b_in": b_in_c, "b_out": b_out_c,
        })
    return meta, in_maps


def _build(meta: Meta):
    nc = bacc.Bacc("TRN2", target_bir_lowering=False, debug=False,
                   enable_asserts=False, num_devices=meta.C,
                   num_swdge_queues=4)
    KA = meta.DA + 1
    T_alloc = meta.T_alloc
    NG = T_alloc // 4
    WIN = meta.WIN
    GK = meta.GK
    H = meta.HALF
    NLO = meta.NLO
    NHI = meta.NP - NLO

    xT_all = nc.dram_tensor("xT_all", [P, meta.NP], BF16, kind="ExternalInput")
    w01_d = nc.dram_tensor("w01", [P, 2 * P], BF16, kind="ExternalInput")
    wabd_d = nc.dram_tensor("wabd", [4 * KA, 4 * P], BF16,
                            kind="ExternalInput")
    iota_d = nc.dram_tensor("iota", [P, WIN], BF16, kind="ExternalInput")
    ident_d = nc.dram_tensor("ident", [P, P], BF16, kind="ExternalInput")
    slabs_d = nc.dram_tensor("slabs", [4 * KA, NG * P], BF16,
                             kind="ExternalInput")
    idx0_d = nc.dram_tensor("idx0", [P, T_alloc * 8], I16,
                            kind="ExternalInput")
    idx1_d = nc.dram_tensor("idx1", [P, T_alloc * 8], I16,
                            kind="ExternalInput")
    dofT_d = nc.dram_tensor("dofT", [P, T_alloc], BF16, kind="ExternalInput")
    xtn_d = nc.dram_tensor("xtn", [P, meta.NW * WIN], F32,
                           kind="ExternalInput")
    w_in_d = nc.dram_tensor("w_in", [P, P], BF16, kind="ExternalInput")
    w_out_d = nc.dram_tensor("w_out", [P, P], BF16, kind="ExternalInput")
    b_in_d = nc.dram_tensor("b_in", [P, 1], F32, kind="ExternalInput")
    b_out_d = nc.dram_tensor("b_out", [P, 1], F32, kind="ExternalInput")

    tab_lo = nc.dram_tensor("tab_lo", [NLO, 2 * P], BF16, kind="Internal")
    tab_hi = nc.dram_tensor("tab_hi", [max(NHI, P), 2 * P], BF16,
                            kind="Internal")
    yT_d = nc.dram_tensor("yT", [P, meta.NW * WIN], F32, kind="ExternalOutput")

    qstate = [0]

    with tile.TileContext(nc) as tc:
        with (
            tc.tile_pool(name="const", bufs=1) as cpool,
            tc.tile_pool(name="xt", bufs=3) as xtp,
            tc.tile_pool(name="tabsb", bufs=3) as tabsb,
            tc.tile_pool(name="slab", bufs=3) as slabp,
            tc.tile_pool(name="hg", bufs=4) as hgp,
            tc.tile_pool(name="add1", bufs=2) as add1p,
            tc.tile_pool(name="msg", bufs=4) as msgp,
            tc.tile_pool(name="oh", bufs=4) as ohp,
            tc.tile_pool(name="mlp", bufs=4) as mlpp,
            tc.tile_pool(name="ps_tab", bufs=2, space="PSUM") as pst,
            tc.tile_pool(name="ps_aemb", bufs=2, space="PSUM") as psa,
            tc.tile_pool(name="ps_agg", bufs=2, space="PSUM") as psg,
            tc.tile_pool(name="ps_mlp", bufs=2, space="PSUM") as psm,
        ):
            idx0 = cpool.tile([P, T_alloc * 8], I16, tag="idx0")
            idx1 = cpool.tile([P, T_alloc * 8], I16, tag="idx1")
            dofT = cpool.tile([P, T_alloc], BF16, tag="dofT")
            iota = cpool.tile([P, WIN], BF16, tag="iota")
            ident = cpool.tile([P, P], BF16, tag="ident")
            w01 = cpool.tile([P, 2 * P], BF16, tag="w01")
            wabd = cpool.tile([4 * KA, 4 * P], BF16, tag="wabd")
            xtn = cpool.tile([P, meta.NW * WIN], F32, tag="xtn")
            w_in = cpool.tile([P, P], BF16, tag="w_in")
            w_out = cpool.tile([P, P], BF16, tag="w_out")
            b_in = cpool.tile([P, 1], F32, tag="b_in")
            b_out = cpool.tile([P, 1], F32, tag="b_out")
            agg_sb = cpool.tile([P, meta.NW * WIN], F32, tag="agg_sb")
            for t, d in [(idx0, idx0_d), (idx1, idx1_d), (dofT, dofT_d),
                         (iota, iota_d), (ident, ident_d),
                         (w01, w01_d), (wabd, wabd_d),
                         (xtn, xtn_d), (w_in, w_in_d), (w_out, w_out_d),
                         (b_in, b_in_d), (b_out, b_out_d)]:
                nc.sync.dma_start(t[:], d[:])

            # ---- phase 1: fused tables, lo half first ----
            def build_tab(i0, i1, dest, dbase):
                # chunks of 2 node-tiles, one write DMA per chunk
                CH = 2
                for i in range(i0, i1, CH):
                    k = min(CH, i1 - i)
                    xt = xtp.tile([P, CH * P], BF16, tag="xt")
                    nc.sync.dma_start(xt[:, :k * P],
                                      xT_all[:, i * P:(i + k) * P])
                    ps = pst.tile([P, CH * 2 * P], F32, tag="pt")
                    for j in range(k):
                        nc.tensor.matmul(ps[:, j * 2 * P:(j + 1) * 2 * P],
                                         xt[:, j * P:(j + 1) * P], w01[:],
                                         start=True, stop=True)
                    hsb = tabsb.tile([P, CH * 2 * P], BF16, tag="hsb")
                    nc.any.tensor_copy(hsb[:, :k * 2 * P], ps[:, :k * 2 * P])
                    r0 = i * P - dbase
                    out3 = dest[r0:r0 + k * P, :].rearrange(
                        "(j p) q -> p j q", p=P)
                    in3 = hsb[:, :k * 2 * P].rearrange("p (j q) -> p j q", j=k)
                    nc.sync.dma_start(out3, in3)

            nlo_t = NLO // P
            build_tab(0, nlo_t, tab_lo, 0)
            build_tab(nlo_t, meta.NT, tab_hi, NLO)

            # ---- phase 2: streams ----
            def finalize(w):
                sl = slice(w * WIN, (w + 1) * WIN)
                hbf = mlpp.tile([P, WIN], BF16, tag="hbf")
                nc.vector.tensor_add(hbf[:], agg_sb[:, sl], xtn[:, sl])
                z1 = psm.tile([P, WIN], F32, tag="pm")
                nc.tensor.matmul(z1[:], w_in[:], hbf[:], start=True, stop=True)
                z1b = mlpp.tile([P, WIN], BF16, tag="z1b")
                nc.scalar.activation(z1b[:], z1[:],
                                     mybir.ActivationFunctionType.Relu,
                                     bias=b_in[:, 0:1])
                z2 = psm.tile([P, WIN], F32, tag="pm")
                nc.tensor.matmul(z2[:], w_out[:], z1b[:], start=True,
                                 stop=True)
                ysb = mlpp.tile([P, WIN], F32, tag="ysb")
                nc.scalar.activation(ysb[:], z2[:],
                                     mybir.ActivationFunctionType.Identity,
                                     bias=b_out[:, 0:1])
                nc.sync.dma_start(yT_d[:, sl], ysb[:])

            live = [s for s in range(4) if meta.TPW[s] > 0]
            base = 0
            for s in range(4):
                Rp = meta.R_pad[s]
                if Rp == 0:
                    continue
                Rr = meta.R[s]
                h0v = (tab_hi if s >= 2 else tab_lo)[:, 0:P]
                h1v = (tab_hi if s % 2 == 1 else tab_lo)[:, P:2 * P]
                first = (s == live[0])
                last = (s == live[-1])
                agg = None
                for c0 in range(0, Rp, GK):
                    k = min(GK, Rp - c0)
                    q0c = (base + c0) * 8
                    hg0 = hgp.tile([P, GK, P], BF16, tag="hg0")
                    nc.gpsimd.dma_gather(
                        out_ap=hg0[:, :k, :], in_ap=h0v,
                        idxs_ap=idx0[:, q0c:q0c + k * 8],
                        num_idxs=k * P, num_idxs_reg=k * P, elem_size=P,
                        elem_step=2 * P, single_packet=False,
                        queue_num=qstate[0] % 4)
                    qstate[0] += 1
                    hg1 = hgp.tile([P, GK, P], BF16, tag="hg1")
                    nc.gpsimd.dma_gather(
                        out_ap=hg1[:, :k, :], in_ap=h1v,
                        idxs_ap=idx1[:, q0c:q0c + k * 8],
                        num_idxs=k * P, num_idxs_reg=k * P, elem_size=P,
                        elem_step=2 * P, single_packet=False,
                        queue_num=qstate[0] % 4)
                    qstate[0] += 1
                    slab = slabp.tile([4 * KA, (GK // 4) * P], BF16,
                                      tag="slab")
                    gf0 = (base + c0) // 4
                    nc.sync.dma_start(slab[:, :(k // 4) * P],
                                      slabs_d[:, gf0 * P:(gf0 + k // 4) * P])
                    add1 = add1p.tile([P, GK * P], BF16, tag="add1")
                    nc.vector.tensor_add(
                        add1[:, :k * P],
                        hg0[:, :k, :].rearrange("p a b -> p (a b)"),
                        hg1[:, :k, :].rearrange("p a b -> p (a b)"))
                    for gi in range(k // 4):
                        aps = psa.tile([P, 4 * P], F32, tag="aemb")
                        nc.tensor.matmul(aps[:], slab[:, gi * P:(gi + 1) * P],
                                         wabd[:], start=True, stop=False)
                        nc.tensor.matmul(aps[:], ident[:],
                                         add1[:, gi * 4 * P:(gi + 1) * 4 * P],
                                         start=False, stop=True)
                        msg = msgp.tile([P, 4 * P], BF16, tag="msg")
                        nc.scalar.activation(msg[:], aps[:],
                                             mybir.ActivationFunctionType.Relu)
                        tau0 = base + c0 + gi * 4
                        oh4 = ohp.tile([P, 4, WIN], BF16, tag="oh4")
                        for u in range(4):
                            nc.vector.tensor_tensor(
                                oh4[:, u, :], iota[:],
                                dofT[:, tau0 + u:tau0 + u + 1]
                                .broadcast_to((P, WIN)),
                                op=mybir.AluOpType.is_equal)
                        for t in range(4):
                            pos = c0 + gi * 4 + t
                            if pos >= Rr:
                                break
                            w, t_in_w = divmod(pos, meta.TPW[s])
                            if t_in_w == 0:
                                agg = psg.tile([P, WIN], F32, tag="agg")
                            nc.tensor.matmul(agg[:],
                                             msg[:, t * P:(t + 1) * P],
                                             oh4[:, t, :],
                                             start=(t_in_w == 0),
                                             stop=(t_in_w == meta.TPW[s] - 1),
                                             skip_group_check=True)
                            if t_in_w == meta.TPW[s] - 1:
                                sl = slice(w * WIN, (w + 1) * WIN)
                                if first:
                                    nc.any.tensor_copy(agg_sb[:, sl], agg[:])
                                else:
                                    nc.vector.tensor_add(agg_sb[:, sl],
                                                         agg_sb[:, sl],
                                                         agg[:])
                                if last:
                                    finalize(w)
                base += Rp

    nc.compile()
    return nc


def run(inputs: dict, C=8, gk=32, half=32768, win=256, trace=False):
    meta, in_maps = _host_prep(
        inputs["x"], inputs["index"], inputs["a"], inputs["W0"], inputs["b0"],
        inputs["W1"], inputs["b1"], inputs["Wa"], inputs["ba"], inputs["eps"],
        inputs["W_in"], inputs["b_in"], inputs["W_out"], inputs["b_out"],
        C=C, gk=gk, half=half, win=win)
    nc = _build(meta)
    res = bass_utils.run_bass_kernel_spmd(nc, in_maps, core_ids=list(range(C)),
                                          trace=trace)
    N = meta.N
    out = np.empty((N, P), np.float32)
    for c in range(C):
        lo = c * meta.NPC
        hi = min((c + 1) * meta.NPC, N)
        out[lo:hi] = res.results[c]["yT"].T[:hi - lo]
    return out, res, meta, in_maps, nc


def kernel(**inputs) -> np.ndarray:
    out, _, _, _, _ = run(inputs)
    return out
